# revision 9
# baseline (speedup 1.0000x reference)
"""APPNP GNN (MLP + K-hop propagation) as a multi-core Bass/Tile kernel for TRN2.

Algorithm (per hop): h <- (1-a) * Ahat @ h + a * h0, Ahat = D^-1/2 (A+I) D^-1/2.

Device strategy (8 cores, SPMD):
  - nodes row-partitioned: core c owns rows [c*R, (c+1)*R), R = nW*128
  - scaled state H' = dinv * h kept in DRAM as bf16 rows padded to 256B
    ([*, 128] bf16, first 64 cols live), replicated via AllGather
  - the per-core slice is split in 4 quarters; each hop runs 4 quarter
    AllGathers so gathers on chunk c start as soon as quarter c arrived
  - per hop, per 128-row dst window: gather H'[src] rows for the window's
    incident edges (dma_gather with 128-byte elems at 256B stride, int16 idx
    bucketed into the 4 chunks), build the window's one-hot S tiles
    [128 edge-slots x 128 dst] in ONE DVE tensor_tensor is_equal
    (iota vs broadcast dstl), segment-sum via PE matmuls accumulating in
    PSUM; the teleport term alpha*h0 enters the PSUM chain as an
    identity-stationary matmul, so the epilogue is a single DVE multiply:
    H'_next = 0.9*dinv^2*(agg + h0pre), h0pre = (alpha/0.9)*sqrt(deg)*h0.
  - edges padded per (window, chunk) bucket to a multiple of 128, sized as the
    max across cores so the program is identical on all cores. Pad slots gather
    row 0 and are killed by dstl = -1 (S row all zero).
"""

import sys
from contextlib import ExitStack
from dataclasses import dataclass

import numpy as np

sys.path.insert(0, "/opt/trn_rl_repo")

import concourse.bacc as bacc
import concourse.bass as bass
import concourse.mybir as mybir
import concourse.tile as tile
from concourse import ap_utils
from concourse.bass import MemorySpace
from concourse._compat import cdiv, exact_div

F32 = mybir.dt.float32
BF16 = mybir.dt.bfloat16
I16 = mybir.dt.int16
AF = mybir.ActivationFunctionType
ALU = mybir.AluOpType

WINDOW = 128
ROWB = 128          # bf16 row stride in elements (256B); first 64 live


def round_up(x, m):
    return (x + m - 1) // m * m


def dma_gather128(gp, out_ap, in_ap, idxs_ap, num_idxs, num_idxs_reg,
                  elem_size, elem_step, queue_num=0, single_packet=True):
    """bass.BassGpSimd.dma_gather with the elem%256B assert relaxed to 128B
    (row stride must still be a 256B multiple)."""
    self = gp
    self._assert_queue_num(queue_num)
    assert idxs_ap.dtype == mybir.dt.int16
    assert in_ap.space == MemorySpace.DRAM
    assert in_ap.dtype == out_ap.dtype
    elem_size_bytes = elem_size * mybir.dt.size(in_ap.dtype)
    assert elem_size_bytes > 0 and elem_size_bytes % 128 == 0
    assert idxs_ap.space == MemorySpace.SBUF
    assert out_ap.space == MemorySpace.SBUF
    assert ap_utils.ap_is_contiguous(in_ap.ap[1:])
    assert ap_utils.ap_is_contiguous(out_ap.ap[1:])
    assert ap_utils.ap_is_contiguous(idxs_ap.ap[1:])
    assert in_ap.ap[-1][1] == out_ap.ap[-1][1] == elem_size
    assert out_ap.ap[0][1] * out_ap.ap[1][1] == round_up(num_idxs, 128)
    assert in_ap.ap[0][0] == elem_step
    stride_bytes = elem_step * mybir.dt.size(in_ap.dtype)
    stride_bytes_256 = exact_div(stride_bytes, 256)
    assert stride_bytes_256 < 256

    _in_ap = self.lower_ap_dma(in_ap, for_custom_bir_dma=True)
    _idxs_ap = self.lower_ap(idxs_ap)
    _out_ap = self.lower_ap(out_ap)
    return self.add_instruction(
        mybir.InstDMAGatherAnt(
            name=self.bass.get_next_instruction_name(),
            ins=[*_in_ap, _idxs_ap,
                 self.lower_val_access(self.to_reg(num_idxs_reg))],
            outs=[_out_ap],
            transpose=False,
            num_idxs=num_idxs,
            elem_size=elem_size,
            stride_bytes_256=stride_bytes_256,
            gen_mode=0,
            single_packet=single_packet,
            queue_num=queue_num,
            sbuf_tokens_per_rank=0,
            sbuf_free_dim_per_rank=0,
            sbuf_free_dim_pad_per_rank=0,
            sbuf_byte_offset=0,
        ))


@dataclass
class Cfg:
    N: int
    E: int          # edges before self loops
    F: int = 512
    H: int = 256
    C: int = 64
    K: int = 10
    alpha: float = 0.1
    n_cores: int = 8
    n_parts: int = 4          # slice quarters == src chunks
    G: int = 5                # windows per gather group (quarter = nW/4/G groups)
    mlp_block: int = 512      # rows per MLP block (<=512)
    max_gather: int = 1024    # per-instruction idx limit (SWDGE ring capacity)
    n_queues: int = 4         # SWDGE queues to rotate gathers across

    @property
    def R(self):  # rows per core: multiple of 128 * n_parts
        q = WINDOW * self.n_parts
        return cdiv(cdiv(self.N, self.n_cores), q) * q

    @property
    def part_rows(self):
        return self.R // self.n_parts

    @property
    def chunk_rows(self):     # rows per AllGather'd chunk (all cores' part p)
        return self.part_rows * self.n_cores

    @property
    def N_pad(self):
        return self.R * self.n_cores

    @property
    def nW(self):
        return self.R // WINDOW

    @property
    def n_chunks(self):
        return self.n_parts

    @property
    def n_groups(self):
        return cdiv(self.nW, self.G)

    def group_windows(self, g):
        return range(g * self.G, min((g + 1) * self.G, self.nW))


@dataclass
class Plan:
    tiles: np.ndarray            # [nW, n_chunks] tiles per bucket
    ng: np.ndarray               # [n_groups, n_chunks] idxs per (g, c) stream
    idx_col_off: np.ndarray      # [n_groups, n_chunks] col offset into idx dram
    gbuf_col_off: np.ndarray     # [nW, n_chunks] tile col within (g,c) gather buf
    bucket_slot_off: np.ndarray  # [nW, n_chunks] slot offset in the stream
    w_tile_off: np.ndarray       # [nW] first dstl tile col of window w
    total_slots: int
    idx_cols_total: int
    dstl_tiles_total: int
    gbuf_tiles_max: np.ndarray   # [n_chunks] max tile count of any (g, c) buf
    T_max: int                   # max tiles of any window


def make_plan(cfg: Cfg, counts_max: np.ndarray) -> Plan:
    padded = (np.ceil(counts_max / WINDOW).astype(np.int64)) * WINDOW
    tiles = padded // WINDOW

    ng = np.zeros((cfg.n_groups, cfg.n_chunks), dtype=np.int64)
    idx_col_off = np.zeros_like(ng)
    gbuf_col_off = np.zeros((cfg.nW, cfg.n_chunks), dtype=np.int64)
    bucket_slot_off = np.zeros_like(gbuf_col_off)

    off = 0
    for g in range(cfg.n_groups):
        for c in range(cfg.n_chunks):
            idx_col_off[g, c] = off // 16
            seg0 = off
            for w in cfg.group_windows(g):
                bucket_slot_off[w, c] = off
                gbuf_col_off[w, c] = (off - seg0) // WINDOW
                off += padded[w, c]
            ng[g, c] = off - seg0

    w_tile_off = np.zeros(cfg.nW, dtype=np.int64)
    t = 0
    for g in range(cfg.n_groups):
        for w in cfg.group_windows(g):
            w_tile_off[w] = t
            t += int(tiles[w, :].sum())

    gmax = np.zeros(cfg.n_chunks, dtype=np.int64)
    for c in range(cfg.n_chunks):
        for g in range(cfg.n_groups):
            s = sum(int(tiles[w, c]) for w in cfg.group_windows(g))
            gmax[c] = max(gmax[c], s)

    return Plan(tiles, ng, idx_col_off, gbuf_col_off, bucket_slot_off,
                w_tile_off, off, off // 16, t,
                gmax, int(tiles.sum(axis=1).max()))


def host_prep(cfg: Cfg, x, W1, b1, W2, b2, edge_index):
    N, R, PR = cfg.N, cfg.R, cfg.part_rows
    src = np.concatenate([edge_index[0], np.arange(N, dtype=np.int64)]).astype(np.int64)
    dst = np.concatenate([edge_index[1], np.arange(N, dtype=np.int64)]).astype(np.int64)

    deg = np.bincount(dst, minlength=N).astype(np.float64)
    dinv = (1.0 / np.sqrt(deg)).astype(np.float32)
    dinv_pad = np.ones(cfg.N_pad, dtype=np.float32)
    dinv_pad[:N] = dinv

    core_of = dst // R
    w_of = (dst % R) // WINDOW
    dstl_rel = (dst % WINDOW).astype(np.float32)
    # chunk p = union over cores of each core's slice quarter p;
    # AllGather_p output position: src_core * PR + (src % R) % PR
    src_off = src % R
    chunk_of = src_off // PR
    idx_local = (src // R) * PR + (src_off % PR)

    nW, nC, nCh = cfg.nW, cfg.n_cores, cfg.n_chunks
    bucket = (core_of * nW + w_of) * nCh + chunk_of
    n_buckets = nC * nW * nCh
    counts = np.bincount(bucket, minlength=n_buckets).reshape(nC, nW, nCh)
    counts_max = counts.max(axis=0)
    plan = make_plan(cfg, counts_max)

    order = np.argsort(bucket, kind="stable")
    sorted_bucket = bucket[order]
    seg_starts = np.searchsorted(sorted_bucket, np.arange(n_buckets))
    rank_sorted = np.arange(len(src)) - seg_starts[sorted_bucket]
    rank = np.empty_like(rank_sorted)
    rank[order] = rank_sorted

    slot_of = plan.bucket_slot_off[w_of, chunk_of] + rank

    deg_sq = np.sqrt(deg).astype(np.float32)

    from ml_dtypes import bfloat16

    prev = np.zeros((nW, nCh), dtype=np.int64)
    cum = np.cumsum(plan.tiles, axis=1)
    prev[:, 1:] = cum[:, :-1]

    def dstl_cols_for(mask):
        out = np.full((plan.dstl_tiles_total, WINDOW), -1.0, dtype=np.float32)
        sl = slot_of[mask]
        w = w_of[mask]
        c = chunk_of[mask]
        rel = sl - plan.bucket_slot_off[w, c]
        t_in_bucket = rel // WINDOW
        p = rel % WINDOW
        col = plan.w_tile_off[w] + prev[w, c] + t_in_bucket
        out[col, p] = dstl_rel[mask]
        return np.ascontiguousarray(out.T).astype(bfloat16)  # [128, tiles]

    in_maps = []
    for core in range(nC):
        xc = np.zeros((R, cfg.F), dtype=np.float32)
        take = min(N - core * R, R)
        xc[:take] = x[core * R: core * R + take]
        xT = np.ascontiguousarray(xc.T).astype(bfloat16)

        mask = core_of == core
        idx_stream = np.zeros(plan.total_slots, dtype=np.int16)
        idx_stream[slot_of[mask]] = idx_local[mask].astype(np.int16)
        idx_w = idx_stream.reshape(-1, 16).T
        idx_rep = np.tile(idx_w, (8, 1)).astype(np.int16)

        dstl_cols = dstl_cols_for(mask)

        dv = dinv_pad[core * R: (core + 1) * R].reshape(nW, WINDOW).T
        rd = np.ones((R,), dtype=np.float32)
        rd[:take] = deg_sq[core * R: core * R + take]
        rd = rd.reshape(nW, WINDOW).T

        iota = np.tile(np.arange(WINDOW, dtype=np.float32), (WINDOW, 1))
        iota_wide = np.tile(iota[:, None, :], (1, plan.T_max, 1))
        eye64 = np.eye(64, dtype=np.float32)
        eye128 = np.eye(128, dtype=np.float32)

        a09 = cfg.alpha / (1.0 - cfg.alpha)

        in_maps.append({
            "xT": xT,
            "W1": W1.astype(bfloat16),
            "b1": b1.reshape(cfg.H, 1).astype(np.float32),
            "W2": W2.astype(bfloat16),
            "b2": b2.reshape(cfg.C, 1).astype(np.float32),
            "iota_wide": iota_wide.astype(bfloat16),
            "eye64": eye64,
            "eye128": eye128.astype(bfloat16),
            "idxs": np.ascontiguousarray(idx_rep),
            "dstl": dstl_cols,
            "dinv_col": np.ascontiguousarray(dv),
            "h0w_col": np.ascontiguousarray(a09 * rd),
            "dinv09sq_col": np.ascontiguousarray((1.0 - cfg.alpha) * dv * dv),
            "rdinv_col": np.ascontiguousarray(rd),
        })
    return in_maps, plan


def build_kernel(cfg: Cfg, plan: Plan):
    nc = bacc.Bacc("TRN2", target_bir_lowering=False, debug=False,
                   num_devices=cfg.n_cores, num_swdge_queues=cfg.n_queues)
    _gq = [0]

    def emit_gather(gb_ap, src_ap, it_ap, ngc):
        o = 0
        while o < ngc:
            n = min(cfg.max_gather, ngc - o)
            dma_gather128(
                nc.gpsimd,
                gb_ap[:, o // 128:(o + n) // 128, :],
                src_ap,
                it_ap[:, o // 16:(o + n) // 16],
                n, n, cfg.C, ROWB,
                queue_num=_gq[0] % cfg.n_queues)
            _gq[0] += 1
            o += n

    R, nW, C, H, F = cfg.R, cfg.nW, cfg.C, cfg.H, cfg.F
    nCh, PR = cfg.n_chunks, cfg.part_rows

    xT_d = nc.dram_tensor("xT", [F, R], BF16, kind="ExternalInput")
    W1_d = nc.dram_tensor("W1", [F, H], BF16, kind="ExternalInput")
    b1_d = nc.dram_tensor("b1", [H, 1], F32, kind="ExternalInput")
    W2_d = nc.dram_tensor("W2", [H, C], BF16, kind="ExternalInput")
    b2_d = nc.dram_tensor("b2", [C, 1], F32, kind="ExternalInput")
    iota_d = nc.dram_tensor("iota_wide", [WINDOW, plan.T_max, WINDOW], BF16,
                            kind="ExternalInput")
    eye64_d = nc.dram_tensor("eye64", [64, 64], F32, kind="ExternalInput")
    eye128_d = nc.dram_tensor("eye128", [128, 128], BF16, kind="ExternalInput")
    idxs_d = nc.dram_tensor("idxs", [128, plan.idx_cols_total], I16,
                            kind="ExternalInput")
    dstl_d = nc.dram_tensor("dstl", [128, plan.dstl_tiles_total], BF16,
                            kind="ExternalInput")
    dinv_d = nc.dram_tensor("dinv_col", [WINDOW, nW], F32, kind="ExternalInput")
    h0w_d = nc.dram_tensor("h0w_col", [WINDOW, nW], F32, kind="ExternalInput")
    d9sq_d = nc.dram_tensor("dinv09sq_col", [WINDOW, nW], F32, kind="ExternalInput")
    rdinv_d = nc.dram_tensor("rdinv_col", [WINDOW, nW], F32, kind="ExternalInput")
    out_d = nc.dram_tensor("out", [R, C], F32, kind="ExternalOutput")

    groups = [list(range(cfg.n_cores))]

    with tile.TileContext(nc) as tc, ExitStack() as st:
        const = st.enter_context(tc.tile_pool(name="const", bufs=1))
        dram = st.enter_context(tc.tile_pool(name="dram", bufs=1, space="DRAM"))

        H_slice = dram.tile([R, ROWB], BF16)
        H_fulls = [[dram.tile([cfg.chunk_rows, ROWB], BF16, addr_space="Shared",
                              tag=f"hfull{k}_{p}", name=f"hfull{k}_{p}")
                    for p in range(cfg.n_parts)]
                   for k in range(cfg.K)]

        iota_sb = const.tile([WINDOW, plan.T_max, WINDOW], BF16, tag="iota")
        nc.sync.dma_start(iota_sb[:], iota_d[:])
        eye64_sb = const.tile([64, 64], F32, tag="eye64")
        nc.sync.dma_start(eye64_sb[:], eye64_d[:])
        eye128_sb = const.tile([128, 128], BF16, tag="eye128")
        nc.sync.dma_start(eye128_sb[:], eye128_d[:])
        dstl_sb = const.tile([128, plan.dstl_tiles_total], BF16, tag="dstl")
        nc.sync.dma_start(dstl_sb[:], dstl_d[:])
        dinv_sb = const.tile([WINDOW, nW], F32, tag="dinv")
        nc.sync.dma_start(dinv_sb[:], dinv_d[:])
        h0w_sb = const.tile([WINDOW, nW], F32, tag="h0w")
        nc.sync.dma_start(h0w_sb[:], h0w_d[:])
        d9sq_sb = const.tile([WINDOW, nW], F32, tag="d9sq")
        nc.sync.dma_start(d9sq_sb[:], d9sq_d[:])
        rdinv_sb = const.tile([WINDOW, nW], F32, tag="rdinv")
        nc.sync.dma_start(rdinv_sb[:], rdinv_d[:])
        h0pre_sb = const.tile([WINDOW, nW, C], BF16, tag="h0pre")

        W1t = []
        for kc in range(F // 128):
            t = const.tile([128, H], BF16, tag=f"w1_{kc}")
            nc.sync.dma_start(t[:], W1_d[kc * 128:(kc + 1) * 128, :])
            W1t.append(t)
        W2t = []
        for kc in range(H // 128):
            t = const.tile([128, C], BF16, tag=f"w2_{kc}")
            nc.sync.dma_start(t[:], W2_d[kc * 128:(kc + 1) * 128, :])
            W2t.append(t)
        b1c = []
        for hh in range(H // 128):
            t = const.tile([128, 1], F32, tag=f"b1_{hh}")
            nc.sync.dma_start(t[:], b1_d[hh * 128:(hh + 1) * 128, :])
            b1c.append(t)
        b2c = const.tile([C, 1], F32, tag="b2")
        nc.sync.dma_start(b2c[:], b2_d[:])

        # ---- phase 1: MLP -> h0pre (SBUF) and H'_0 -> H_slice (DRAM)
        with tc.tile_pool(name="mlp", bufs=3) as mp, \
             tc.tile_pool(name="mlp_ps", bufs=2, space="PSUM") as pp1, \
             tc.tile_pool(name="mlp_ps2", bufs=2, space="PSUM") as pp2, \
             tc.tile_pool(name="mlp_pst", bufs=2, space="PSUM") as ppt:
            r0 = 0
            while r0 < R:
                B = min(cfg.mlp_block, R - r0)
                xt = []
                for kc in range(F // 128):
                    t = mp.tile([128, cfg.mlp_block], BF16, tag=f"x_{kc}")
                    nc.sync.dma_start(t[:, :B], xT_d[kc * 128:(kc + 1) * 128, r0:r0 + B])
                    xt.append(t)
                h1 = []
                for half in range(H // 128):
                    ps = pp1.tile([128, cfg.mlp_block], F32, tag=f"ps1_{half}")
                    for kc in range(F // 128):
                        nc.tensor.matmul(
                            ps[:, :B],
                            W1t[kc][:, half * 128:(half + 1) * 128],
                            xt[kc][:, :B],
                            start=(kc == 0), stop=(kc == F // 128 - 1))
                    h = mp.tile([128, cfg.mlp_block], BF16, tag=f"h1_{half}")
                    nc.scalar.activation(h[:, :B], ps[:, :B], AF.Relu, bias=b1c[half][:])
                    h1.append(h)
                ps2 = pp2.tile([C, cfg.mlp_block], F32, tag="ps2")
                for kc in range(H // 128):
                    nc.tensor.matmul(ps2[:, :B], W2t[kc][:], h1[kc][:, :B],
                                     start=(kc == 0), stop=(kc == H // 128 - 1))
                hT = mp.tile([C, cfg.mlp_block], F32, tag="hT")
                nc.scalar.activation(hT[:, :B], ps2[:, :B], AF.Identity, bias=b2c[:])
                for j in range(B // WINDOW):
                    w = (r0 // WINDOW) + j
                    pst = ppt.tile([WINDOW, C], F32, tag="pst")
                    nc.tensor.transpose(pst[:], hT[:, j * WINDOW:(j + 1) * WINDOW], eye64_sb[:])
                    nc.vector.tensor_scalar_mul(h0pre_sb[:, w, :], pst[:],
                                                h0w_sb[:, w:w + 1])
                    hp = mp.tile([WINDOW, C], BF16, tag="hp")
                    nc.vector.tensor_scalar_mul(hp[:], pst[:], dinv_sb[:, w:w + 1])
                    nc.sync.dma_start(
                        H_slice[w * WINDOW:(w + 1) * WINDOW, :C], hp[:])
                r0 += B

        # ---- phase 2: K propagation hops
        hop_pools = {
            "idx": st.enter_context(tc.tile_pool(name="idx", bufs=3)),
            "gb": st.enter_context(tc.tile_pool(name="gb", bufs=2)),
            "S": st.enter_context(tc.tile_pool(name="S", bufs=cfg.G + 1)),
            "hw": st.enter_context(tc.tile_pool(name="hw", bufs=4)),
            "ps": st.enter_context(tc.tile_pool(name="ps", bufs=4, space="PSUM")),
        }

        def emit_allgather(HF_next, p):
            nc.gpsimd.collective_compute(
                "AllGather", ALU.bypass, replica_groups=groups,
                ins=[H_slice[p * PR:(p + 1) * PR, :].opt()],
                outs=[HF_next[p].opt()])

        def hop_body(HF, HF_next):
            # HF was AllGather'd during the previous hop; fire the NEXT hop's
            # quarter-AllGathers as soon as each slice quarter is written so
            # the collective overlaps this hop's remaining compute.
            gmax_cols = int(plan.ng.sum(axis=1).max() // 16)
            groups_per_part = cfg.n_groups // cfg.n_parts
            for g in range(cfg.n_groups):
                gsum = int(plan.ng[g, :].sum())
                icol0 = int(plan.idx_col_off[g, 0])
                it_g = hop_pools["idx"].tile([128, gmax_cols], I16, tag="idxg")
                nc.sync.dma_start(it_g[:, :gsum // 16],
                                  idxs_d[:, icol0:icol0 + gsum // 16])
                gbufs = {}
                for c in range(nCh):
                    ngc = int(plan.ng[g, c])
                    if ngc == 0:
                        continue
                    ioff = int(plan.idx_col_off[g, c]) - icol0
                    gb = hop_pools["gb"].tile(
                        [128, int(plan.gbuf_tiles_max[c]), C], BF16, tag=f"gb{c}")
                    emit_gather(gb, HF[c][:, :C],
                                it_g[:, ioff:ioff + ngc // 16], ngc)
                    gbufs[c] = gb
                Ss = {}
                for w in cfg.group_windows(g):
                    T_w = int(plan.tiles[w, :].sum())
                    if T_w == 0:
                        continue
                    S = hop_pools["S"].tile([128, plan.T_max, 128], BF16, tag="S")
                    d0 = int(plan.w_tile_off[w])
                    bc = dstl_sb[:, d0:d0 + T_w, None].broadcast_to((128, T_w, 128))
                    nc.vector.tensor_tensor(S[:, :T_w, :], iota_sb[:, :T_w, :],
                                            bc, op=ALU.is_equal)
                    Ss[w] = S
                for w in cfg.group_windows(g):
                    T_w = int(plan.tiles[w, :].sum())
                    ps = hop_pools["ps"].tile([WINDOW, C], F32, tag="agg")
                    nc.tensor.matmul(ps[:], eye128_sb[:], h0pre_sb[:, w, :],
                                     start=True, stop=(T_w == 0))
                    done = 0
                    for c in range(nCh):
                        T = int(plan.tiles[w, c])
                        for t in range(T):
                            q = int(plan.gbuf_col_off[w, c]) + t
                            nc.tensor.matmul(
                                ps[:], Ss[w][:, done, :], gbufs[c][:, q, :],
                                start=False, stop=(done == T_w - 1))
                            done += 1
                    hp = hop_pools["hw"].tile([WINDOW, C], BF16, tag="hp2")
                    nc.vector.tensor_scalar_mul(hp[:], ps[:], d9sq_sb[:, w:w + 1])
                    nc.sync.dma_start(
                        H_slice[w * WINDOW:(w + 1) * WINDOW, :C], hp[:])
                if HF_next is not None and (g + 1) % groups_per_part == 0:
                    emit_allgather(HF_next, g // groups_per_part)

        for p in range(cfg.n_parts):
            emit_allgather(H_fulls[0], p)
        for k in range(cfg.K):
            hop_body(H_fulls[k], H_fulls[k + 1] if k + 1 < cfg.K else None)

        # ---- phase 3: log_softmax
        with tc.tile_pool(name="sm", bufs=4) as smp, \
             tc.tile_pool(name="smc", bufs=4) as smc:
            for w in range(nW):
                hp = smp.tile([WINDOW, C], BF16, tag="hp3")
                nc.sync.dma_start(hp[:], H_slice[w * WINDOW:(w + 1) * WINDOW, :C])
                h = smp.tile([WINDOW, C], F32, tag="h3")
                nc.vector.tensor_scalar_mul(h[:], hp[:], rdinv_sb[:, w:w + 1])
                nm = smc.tile([WINDOW, 1], F32, tag="nm")
                nc.vector.tensor_reduce(nm[:], h[:], mybir.AxisListType.X,
                                        ALU.max, negate=True)
                e = smp.tile([WINDOW, C], F32, tag="e3")
                se = smc.tile([WINDOW, 1], F32, tag="se")
                nc.scalar.activation(e[:], h[:], AF.Exp, bias=nm[:], accum_out=se[:])
                ls = smc.tile([WINDOW, 1], F32, tag="ls")
                nc.scalar.activation(ls[:], se[:], AF.Ln)
                o = smp.tile([WINDOW, C], F32, tag="o3")
                nc.vector.tensor_scalar(o[:], h[:], nm[:], ls[:],
                                        op0=ALU.add, op1=ALU.subtract)
                nc.sync.dma_start(out_d[w * WINDOW:(w + 1) * WINDOW, :], o[:])

    nc.compile()
    return nc


# test-harness knobs (not used by the grading path, which calls kernel() only)
PROFILE = False
LAST_EXEC_NS = None
LAST_TRACE = None

_BUILD_CACHE: dict = {}


def _get_compiled(cfg: Cfg, plan: Plan):
    key = (cfg.N, cfg.E, cfg.K, cfg.G, plan.tiles.tobytes())
    hit = _BUILD_CACHE.get(key)
    if hit is None:
        hit = build_kernel(cfg, plan)
        _BUILD_CACHE.clear()
        _BUILD_CACHE[key] = hit
    return hit


def kernel(x, W1, b1, W2, b2, edge_index):
    """Full (unsharded) inputs in, full [N, 64] log-softmax output out."""
    from concourse.bass_utils import run_bass_kernel_spmd

    x = np.asarray(x, dtype=np.float32)
    W1 = np.asarray(W1, dtype=np.float32)
    b1 = np.asarray(b1, dtype=np.float32)
    W2 = np.asarray(W2, dtype=np.float32)
    b2 = np.asarray(b2, dtype=np.float32)
    edge_index = np.asarray(edge_index)

    N, F = x.shape
    H = W1.shape[1]
    C = W2.shape[1]
    E = edge_index.shape[1]
    cfg = Cfg(N=N, E=E, F=F, H=H, C=C, K=10, alpha=0.1, n_cores=8)

    in_maps, plan = host_prep(cfg, x, W1, b1, W2, b2, edge_index)
    nc = _get_compiled(cfg, plan)
    res = run_bass_kernel_spmd(nc, in_maps, list(range(cfg.n_cores)),
                               trace=PROFILE)
    if PROFILE:
        global LAST_EXEC_NS, LAST_TRACE
        LAST_EXEC_NS = res.exec_time_ns
        LAST_TRACE = (res.instructions_and_trace or (None, None))[1]
    out = np.concatenate([res.results[i]["out"] for i in range(cfg.n_cores)],
                         axis=0)[:N]
    return np.ascontiguousarray(out, dtype=np.float32)


# revision 14
# speedup vs baseline: 1.0928x; 1.0928x over previous
"""APPNP GNN (MLP + K-hop propagation) as a multi-core Bass/Tile kernel for TRN2.

Algorithm (per hop): h <- (1-a) * Ahat @ h + a * h0, Ahat = D^-1/2 (A+I) D^-1/2.

Device strategy (8 cores, SPMD):
  - nodes row-partitioned: core c owns rows [c*R, (c+1)*R), R = nW*128
  - scaled state H' = dinv * h kept in DRAM as bf16 rows padded to 256B
    ([*, 128] bf16, first 64 cols live), replicated via AllGather
  - the per-core slice is split in 4 quarters; each hop runs 4 quarter
    AllGathers so gathers on chunk c start as soon as quarter c arrived
  - per hop, per 128-row dst window: gather H'[src] rows for the window's
    incident edges (dma_gather with 128-byte elems at 256B stride, int16 idx
    bucketed into the 4 chunks), build the window's one-hot S tiles
    [128 edge-slots x 128 dst] in ONE DVE tensor_tensor is_equal
    (iota vs broadcast dstl), segment-sum via PE matmuls accumulating in
    PSUM; the teleport term alpha*h0 enters the PSUM chain as an
    identity-stationary matmul, so the epilogue is a single DVE multiply:
    H'_next = 0.9*dinv^2*(agg + h0pre), h0pre = (alpha/0.9)*sqrt(deg)*h0.
  - edges padded per (window, chunk) bucket to a multiple of 128, sized as the
    max across cores so the program is identical on all cores. Pad slots gather
    row 0 and are killed by dstl = -1 (S row all zero).
"""

import sys
from contextlib import ExitStack
from dataclasses import dataclass

import numpy as np

sys.path.insert(0, "/opt/trn_rl_repo")

import concourse.bacc as bacc
import concourse.bass as bass
import concourse.mybir as mybir
import concourse.tile as tile
from concourse import ap_utils
from concourse.bass import MemorySpace
from concourse._compat import cdiv, exact_div

F32 = mybir.dt.float32
BF16 = mybir.dt.bfloat16
I16 = mybir.dt.int16
AF = mybir.ActivationFunctionType
ALU = mybir.AluOpType

WINDOW = 128
ROWB = 128          # bf16 row stride in elements (256B); first 64 live


def round_up(x, m):
    return (x + m - 1) // m * m


def dma_gather128(gp, out_ap, in_ap, idxs_ap, num_idxs, num_idxs_reg,
                  elem_size, elem_step, queue_num=0, single_packet=True):
    """bass.BassGpSimd.dma_gather with the elem%256B assert relaxed to 128B
    (row stride must still be a 256B multiple)."""
    self = gp
    self._assert_queue_num(queue_num)
    assert idxs_ap.dtype == mybir.dt.int16
    assert in_ap.space == MemorySpace.DRAM
    assert in_ap.dtype == out_ap.dtype
    elem_size_bytes = elem_size * mybir.dt.size(in_ap.dtype)
    assert elem_size_bytes > 0 and elem_size_bytes % 128 == 0
    assert idxs_ap.space == MemorySpace.SBUF
    assert out_ap.space == MemorySpace.SBUF
    assert ap_utils.ap_is_contiguous(in_ap.ap[1:])
    assert ap_utils.ap_is_contiguous(out_ap.ap[1:])
    assert ap_utils.ap_is_contiguous(idxs_ap.ap[1:])
    assert in_ap.ap[-1][1] == out_ap.ap[-1][1] == elem_size
    assert out_ap.ap[0][1] * out_ap.ap[1][1] == round_up(num_idxs, 128)
    assert in_ap.ap[0][0] == elem_step
    stride_bytes = elem_step * mybir.dt.size(in_ap.dtype)
    stride_bytes_256 = exact_div(stride_bytes, 256)
    assert stride_bytes_256 < 256

    _in_ap = self.lower_ap_dma(in_ap, for_custom_bir_dma=True)
    _idxs_ap = self.lower_ap(idxs_ap)
    _out_ap = self.lower_ap(out_ap)
    return self.add_instruction(
        mybir.InstDMAGatherAnt(
            name=self.bass.get_next_instruction_name(),
            ins=[*_in_ap, _idxs_ap,
                 self.lower_val_access(self.to_reg(num_idxs_reg))],
            outs=[_out_ap],
            transpose=False,
            num_idxs=num_idxs,
            elem_size=elem_size,
            stride_bytes_256=stride_bytes_256,
            gen_mode=0,
            single_packet=single_packet,
            queue_num=queue_num,
            sbuf_tokens_per_rank=0,
            sbuf_free_dim_per_rank=0,
            sbuf_free_dim_pad_per_rank=0,
            sbuf_byte_offset=0,
        ))


@dataclass
class Cfg:
    N: int
    E: int          # edges before self loops
    F: int = 512
    H: int = 256
    C: int = 64
    K: int = 10
    alpha: float = 0.1
    n_cores: int = 8
    n_parts: int = 4          # slice quarters == src chunks
    G: int = 5                # windows per gather group (quarter = nW/4/G groups)
    mlp_block: int = 512      # rows per MLP block (<=512)
    max_gather: int = 4096    # per-instruction idx limit (SWDGE ring capacity
                              # in packets; single_packet=False packs ~32
                              # 128B descriptors per packet)
    n_queues: int = 4         # SWDGE queues to rotate gathers across

    @property
    def R(self):  # rows per core: multiple of 128 * n_parts
        q = WINDOW * self.n_parts
        return cdiv(cdiv(self.N, self.n_cores), q) * q

    @property
    def part_rows(self):
        return self.R // self.n_parts

    @property
    def chunk_rows(self):     # rows per AllGather'd chunk (all cores' part p)
        return self.part_rows * self.n_cores

    @property
    def N_pad(self):
        return self.R * self.n_cores

    @property
    def nW(self):
        return self.R // WINDOW

    @property
    def n_chunks(self):
        return self.n_parts

    @property
    def n_groups(self):
        return cdiv(self.nW, self.G)

    def group_windows(self, g):
        return range(g * self.G, min((g + 1) * self.G, self.nW))


@dataclass
class Plan:
    tiles: np.ndarray            # [nW, n_chunks] tiles per bucket
    ng: np.ndarray               # [n_groups, n_chunks] idxs per (g, c) stream
    idx_col_off: np.ndarray      # [n_groups, n_chunks] col offset into idx dram
    gbuf_col_off: np.ndarray     # [nW, n_chunks] tile col within (g,c) gather buf
    bucket_slot_off: np.ndarray  # [nW, n_chunks] slot offset in the stream
    w_tile_off: np.ndarray       # [nW] first dstl tile col of window w
    total_slots: int
    idx_cols_total: int
    dstl_tiles_total: int
    gbuf_tiles_max: np.ndarray   # [n_chunks] max tile count of any (g, c) buf
    T_max: int                   # max tiles of any window


def make_plan(cfg: Cfg, counts_max: np.ndarray) -> Plan:
    padded = (np.ceil(counts_max / WINDOW).astype(np.int64)) * WINDOW
    tiles = padded // WINDOW

    ng = np.zeros((cfg.n_groups, cfg.n_chunks), dtype=np.int64)
    idx_col_off = np.zeros_like(ng)
    gbuf_col_off = np.zeros((cfg.nW, cfg.n_chunks), dtype=np.int64)
    bucket_slot_off = np.zeros_like(gbuf_col_off)

    off = 0
    for g in range(cfg.n_groups):
        for c in range(cfg.n_chunks):
            idx_col_off[g, c] = off // 16
            seg0 = off
            for w in cfg.group_windows(g):
                bucket_slot_off[w, c] = off
                gbuf_col_off[w, c] = (off - seg0) // WINDOW
                off += padded[w, c]
            ng[g, c] = off - seg0

    w_tile_off = np.zeros(cfg.nW, dtype=np.int64)
    t = 0
    for g in range(cfg.n_groups):
        for w in cfg.group_windows(g):
            w_tile_off[w] = t
            t += int(tiles[w, :].sum())

    gmax = np.zeros(cfg.n_chunks, dtype=np.int64)
    for c in range(cfg.n_chunks):
        for g in range(cfg.n_groups):
            s = sum(int(tiles[w, c]) for w in cfg.group_windows(g))
            gmax[c] = max(gmax[c], s)

    return Plan(tiles, ng, idx_col_off, gbuf_col_off, bucket_slot_off,
                w_tile_off, off, off // 16, t,
                gmax, int(tiles.sum(axis=1).max()))


def host_prep(cfg: Cfg, x, W1, b1, W2, b2, edge_index):
    N, R, PR = cfg.N, cfg.R, cfg.part_rows
    src = np.concatenate([edge_index[0], np.arange(N, dtype=np.int64)]).astype(np.int64)
    dst = np.concatenate([edge_index[1], np.arange(N, dtype=np.int64)]).astype(np.int64)

    deg = np.bincount(dst, minlength=N).astype(np.float64)
    dinv = (1.0 / np.sqrt(deg)).astype(np.float32)
    dinv_pad = np.ones(cfg.N_pad, dtype=np.float32)
    dinv_pad[:N] = dinv

    core_of = dst // R
    w_of = (dst % R) // WINDOW
    dstl_rel = (dst % WINDOW).astype(np.float32)
    # chunk p = union over cores of each core's slice quarter p;
    # AllGather_p output position: src_core * PR + (src % R) % PR
    src_off = src % R
    chunk_of = src_off // PR
    idx_local = (src // R) * PR + (src_off % PR)

    nW, nC, nCh = cfg.nW, cfg.n_cores, cfg.n_chunks
    bucket = (core_of * nW + w_of) * nCh + chunk_of
    n_buckets = nC * nW * nCh
    counts = np.bincount(bucket, minlength=n_buckets).reshape(nC, nW, nCh)
    counts_max = counts.max(axis=0)
    plan = make_plan(cfg, counts_max)

    order = np.argsort(bucket, kind="stable")
    sorted_bucket = bucket[order]
    seg_starts = np.searchsorted(sorted_bucket, np.arange(n_buckets))
    rank_sorted = np.arange(len(src)) - seg_starts[sorted_bucket]
    rank = np.empty_like(rank_sorted)
    rank[order] = rank_sorted

    slot_of = plan.bucket_slot_off[w_of, chunk_of] + rank

    deg_sq = np.sqrt(deg).astype(np.float32)

    from ml_dtypes import bfloat16

    prev = np.zeros((nW, nCh), dtype=np.int64)
    cum = np.cumsum(plan.tiles, axis=1)
    prev[:, 1:] = cum[:, :-1]

    def dstl_cols_for(mask):
        out = np.full((plan.dstl_tiles_total, WINDOW), -1.0, dtype=np.float32)
        sl = slot_of[mask]
        w = w_of[mask]
        c = chunk_of[mask]
        rel = sl - plan.bucket_slot_off[w, c]
        t_in_bucket = rel // WINDOW
        p = rel % WINDOW
        col = plan.w_tile_off[w] + prev[w, c] + t_in_bucket
        out[col, p] = dstl_rel[mask]
        return np.ascontiguousarray(out.T).astype(bfloat16)  # [128, tiles]

    in_maps = []
    for core in range(nC):
        xc = np.zeros((R, cfg.F), dtype=np.float32)
        take = min(N - core * R, R)
        xc[:take] = x[core * R: core * R + take]
        xT = np.ascontiguousarray(xc.T).astype(bfloat16)

        mask = core_of == core
        idx_stream = np.zeros(plan.total_slots, dtype=np.int16)
        idx_stream[slot_of[mask]] = idx_local[mask].astype(np.int16)
        idx_w = idx_stream.reshape(-1, 16).T
        idx_rep = np.tile(idx_w, (8, 1)).astype(np.int16)

        dstl_cols = dstl_cols_for(mask)

        dv = dinv_pad[core * R: (core + 1) * R].reshape(nW, WINDOW).T
        rd = np.ones((R,), dtype=np.float32)
        rd[:take] = deg_sq[core * R: core * R + take]
        rd = rd.reshape(nW, WINDOW).T

        iota = np.tile(np.arange(WINDOW, dtype=np.float32), (WINDOW, 1))
        iota_wide = np.tile(iota[:, None, :], (1, plan.T_max, 1))
        eye64 = np.eye(64, dtype=np.float32)
        eye128 = np.eye(128, dtype=np.float32)

        a09 = cfg.alpha / (1.0 - cfg.alpha)

        in_maps.append({
            "xT": xT,
            "W1": W1.astype(bfloat16),
            "b1": b1.reshape(cfg.H, 1).astype(np.float32),
            "W2": W2.astype(bfloat16),
            "b2": b2.reshape(cfg.C, 1).astype(np.float32),
            "iota_wide": iota_wide.astype(bfloat16),
            "eye64": eye64,
            "eye128": eye128.astype(bfloat16),
            "idxs": np.ascontiguousarray(idx_rep),
            "dstl": dstl_cols,
            "dinv_col": np.ascontiguousarray(dv),
            "h0w_col": np.ascontiguousarray(a09 * rd),
            "dinv09sq_col": np.ascontiguousarray((1.0 - cfg.alpha) * dv * dv),
            "rdinv_col": np.ascontiguousarray(rd),
        })
    return in_maps, plan


def build_kernel(cfg: Cfg, plan: Plan):
    nc = bacc.Bacc("TRN2", target_bir_lowering=False, debug=False,
                   num_devices=cfg.n_cores, num_swdge_queues=cfg.n_queues)
    _gq = [0]

    def emit_gather(gb_ap, src_ap, it_ap, ngc):
        o = 0
        while o < ngc:
            n = min(cfg.max_gather, ngc - o)
            dma_gather128(
                nc.gpsimd,
                gb_ap[:, o // 128:(o + n) // 128, :],
                src_ap,
                it_ap[:, o // 16:(o + n) // 16],
                n, n, cfg.C, ROWB,
                queue_num=_gq[0] % cfg.n_queues,
                single_packet=False)
            _gq[0] += 1
            o += n

    R, nW, C, H, F = cfg.R, cfg.nW, cfg.C, cfg.H, cfg.F
    nCh, PR = cfg.n_chunks, cfg.part_rows

    xT_d = nc.dram_tensor("xT", [F, R], BF16, kind="ExternalInput")
    W1_d = nc.dram_tensor("W1", [F, H], BF16, kind="ExternalInput")
    b1_d = nc.dram_tensor("b1", [H, 1], F32, kind="ExternalInput")
    W2_d = nc.dram_tensor("W2", [H, C], BF16, kind="ExternalInput")
    b2_d = nc.dram_tensor("b2", [C, 1], F32, kind="ExternalInput")
    iota_d = nc.dram_tensor("iota_wide", [WINDOW, plan.T_max, WINDOW], BF16,
                            kind="ExternalInput")
    eye64_d = nc.dram_tensor("eye64", [64, 64], F32, kind="ExternalInput")
    eye128_d = nc.dram_tensor("eye128", [128, 128], BF16, kind="ExternalInput")
    idxs_d = nc.dram_tensor("idxs", [128, plan.idx_cols_total], I16,
                            kind="ExternalInput")
    dstl_d = nc.dram_tensor("dstl", [128, plan.dstl_tiles_total], BF16,
                            kind="ExternalInput")
    dinv_d = nc.dram_tensor("dinv_col", [WINDOW, nW], F32, kind="ExternalInput")
    h0w_d = nc.dram_tensor("h0w_col", [WINDOW, nW], F32, kind="ExternalInput")
    d9sq_d = nc.dram_tensor("dinv09sq_col", [WINDOW, nW], F32, kind="ExternalInput")
    rdinv_d = nc.dram_tensor("rdinv_col", [WINDOW, nW], F32, kind="ExternalInput")
    out_d = nc.dram_tensor("out", [R, C], F32, kind="ExternalOutput")

    groups = [list(range(cfg.n_cores))]

    with tile.TileContext(nc) as tc, ExitStack() as st:
        const = st.enter_context(tc.tile_pool(name="const", bufs=1))
        dram = st.enter_context(tc.tile_pool(name="dram", bufs=1, space="DRAM"))

        H_slice = dram.tile([R, ROWB], BF16)
        H_fulls = [[dram.tile([cfg.chunk_rows, ROWB], BF16, addr_space="Shared",
                              tag=f"hfull{k}_{p}", name=f"hfull{k}_{p}")
                    for p in range(cfg.n_parts)]
                   for k in range(cfg.K)]

        iota_sb = const.tile([WINDOW, plan.T_max, WINDOW], BF16, tag="iota")
        nc.sync.dma_start(iota_sb[:], iota_d[:])
        eye64_sb = const.tile([64, 64], F32, tag="eye64")
        nc.sync.dma_start(eye64_sb[:], eye64_d[:])
        eye128_sb = const.tile([128, 128], BF16, tag="eye128")
        nc.sync.dma_start(eye128_sb[:], eye128_d[:])
        dstl_sb = const.tile([128, plan.dstl_tiles_total], BF16, tag="dstl")
        nc.sync.dma_start(dstl_sb[:], dstl_d[:])
        dinv_sb = const.tile([WINDOW, nW], F32, tag="dinv")
        nc.sync.dma_start(dinv_sb[:], dinv_d[:])
        h0w_sb = const.tile([WINDOW, nW], F32, tag="h0w")
        nc.sync.dma_start(h0w_sb[:], h0w_d[:])
        d9sq_sb = const.tile([WINDOW, nW], F32, tag="d9sq")
        nc.sync.dma_start(d9sq_sb[:], d9sq_d[:])
        rdinv_sb = const.tile([WINDOW, nW], F32, tag="rdinv")
        nc.sync.dma_start(rdinv_sb[:], rdinv_d[:])
        h0pre_sb = const.tile([WINDOW, nW, C], BF16, tag="h0pre")

        W1t = []
        for kc in range(F // 128):
            t = const.tile([128, H], BF16, tag=f"w1_{kc}")
            nc.sync.dma_start(t[:], W1_d[kc * 128:(kc + 1) * 128, :])
            W1t.append(t)
        W2t = []
        for kc in range(H // 128):
            t = const.tile([128, C], BF16, tag=f"w2_{kc}")
            nc.sync.dma_start(t[:], W2_d[kc * 128:(kc + 1) * 128, :])
            W2t.append(t)
        b1c = []
        for hh in range(H // 128):
            t = const.tile([128, 1], F32, tag=f"b1_{hh}")
            nc.sync.dma_start(t[:], b1_d[hh * 128:(hh + 1) * 128, :])
            b1c.append(t)
        b2c = const.tile([C, 1], F32, tag="b2")
        nc.sync.dma_start(b2c[:], b2_d[:])

        # ---- phase 1: MLP -> h0pre (SBUF) and H'_0 -> H_slice (DRAM)
        with tc.tile_pool(name="mlp", bufs=3) as mp, \
             tc.tile_pool(name="mlp_ps", bufs=2, space="PSUM") as pp1, \
             tc.tile_pool(name="mlp_ps2", bufs=2, space="PSUM") as pp2, \
             tc.tile_pool(name="mlp_pst", bufs=2, space="PSUM") as ppt:
            r0 = 0
            while r0 < R:
                B = min(cfg.mlp_block, R - r0)
                xt = []
                for kc in range(F // 128):
                    t = mp.tile([128, cfg.mlp_block], BF16, tag=f"x_{kc}")
                    nc.sync.dma_start(t[:, :B], xT_d[kc * 128:(kc + 1) * 128, r0:r0 + B])
                    xt.append(t)
                h1 = []
                for half in range(H // 128):
                    ps = pp1.tile([128, cfg.mlp_block], F32, tag=f"ps1_{half}")
                    for kc in range(F // 128):
                        nc.tensor.matmul(
                            ps[:, :B],
                            W1t[kc][:, half * 128:(half + 1) * 128],
                            xt[kc][:, :B],
                            start=(kc == 0), stop=(kc == F // 128 - 1))
                    h = mp.tile([128, cfg.mlp_block], BF16, tag=f"h1_{half}")
                    nc.scalar.activation(h[:, :B], ps[:, :B], AF.Relu, bias=b1c[half][:])
                    h1.append(h)
                ps2 = pp2.tile([C, cfg.mlp_block], F32, tag="ps2")
                for kc in range(H // 128):
                    nc.tensor.matmul(ps2[:, :B], W2t[kc][:], h1[kc][:, :B],
                                     start=(kc == 0), stop=(kc == H // 128 - 1))
                hT = mp.tile([C, cfg.mlp_block], F32, tag="hT")
                nc.scalar.activation(hT[:, :B], ps2[:, :B], AF.Identity, bias=b2c[:])
                for j in range(B // WINDOW):
                    w = (r0 // WINDOW) + j
                    pst = ppt.tile([WINDOW, C], F32, tag="pst")
                    nc.tensor.transpose(pst[:], hT[:, j * WINDOW:(j + 1) * WINDOW], eye64_sb[:])
                    nc.vector.tensor_scalar_mul(h0pre_sb[:, w, :], pst[:],
                                                h0w_sb[:, w:w + 1])
                    hp = mp.tile([WINDOW, C], BF16, tag="hp")
                    nc.vector.tensor_scalar_mul(hp[:], pst[:], dinv_sb[:, w:w + 1])
                    nc.sync.dma_start(
                        H_slice[w * WINDOW:(w + 1) * WINDOW, :C], hp[:])
                r0 += B

        # ---- phase 2: K propagation hops
        hop_pools = {
            "idx": st.enter_context(tc.tile_pool(name="idx", bufs=3)),
            "gb": st.enter_context(tc.tile_pool(name="gb", bufs=2)),
            "S": st.enter_context(tc.tile_pool(name="S", bufs=cfg.G + 1)),
            "hw": st.enter_context(tc.tile_pool(name="hw", bufs=4)),
            "ps": st.enter_context(tc.tile_pool(name="ps", bufs=4, space="PSUM")),
        }

        def emit_allgather(HF_next, p):
            nc.gpsimd.collective_compute(
                "AllGather", ALU.bypass, replica_groups=groups,
                ins=[H_slice[p * PR:(p + 1) * PR, :].opt()],
                outs=[HF_next[p].opt()])

        def hop_body(HF, HF_next):
            # HF was AllGather'd during the previous hop; fire the NEXT hop's
            # quarter-AllGathers as soon as each slice quarter is written so
            # the collective overlaps this hop's remaining compute.
            gmax_cols = int(plan.ng.sum(axis=1).max() // 16)
            groups_per_part = cfg.n_groups // cfg.n_parts
            for g in range(cfg.n_groups):
                gsum = int(plan.ng[g, :].sum())
                icol0 = int(plan.idx_col_off[g, 0])
                it_g = hop_pools["idx"].tile([128, gmax_cols], I16, tag="idxg")
                nc.sync.dma_start(it_g[:, :gsum // 16],
                                  idxs_d[:, icol0:icol0 + gsum // 16])
                gbufs = {}
                for c in range(nCh):
                    ngc = int(plan.ng[g, c])
                    if ngc == 0:
                        continue
                    ioff = int(plan.idx_col_off[g, c]) - icol0
                    gb = hop_pools["gb"].tile(
                        [128, int(plan.gbuf_tiles_max[c]), C], BF16, tag=f"gb{c}")
                    emit_gather(gb, HF[c][:, :C],
                                it_g[:, ioff:ioff + ngc // 16], ngc)
                    gbufs[c] = gb
                Ss = {}
                for w in cfg.group_windows(g):
                    T_w = int(plan.tiles[w, :].sum())
                    if T_w == 0:
                        continue
                    S = hop_pools["S"].tile([128, plan.T_max, 128], BF16, tag="S")
                    d0 = int(plan.w_tile_off[w])
                    bc = dstl_sb[:, d0:d0 + T_w, None].broadcast_to((128, T_w, 128))
                    nc.vector.tensor_tensor(S[:, :T_w, :], iota_sb[:, :T_w, :],
                                            bc, op=ALU.is_equal)
                    Ss[w] = S
                for w in cfg.group_windows(g):
                    T_w = int(plan.tiles[w, :].sum())
                    ps = hop_pools["ps"].tile([WINDOW, C], F32, tag="agg")
                    nc.tensor.matmul(ps[:], eye128_sb[:], h0pre_sb[:, w, :],
                                     start=True, stop=(T_w == 0))
                    done = 0
                    for c in range(nCh):
                        T = int(plan.tiles[w, c])
                        for t in range(T):
                            q = int(plan.gbuf_col_off[w, c]) + t
                            nc.tensor.matmul(
                                ps[:], Ss[w][:, done, :], gbufs[c][:, q, :],
                                start=False, stop=(done == T_w - 1))
                            done += 1
                    hp = hop_pools["hw"].tile([WINDOW, C], BF16, tag="hp2")
                    nc.vector.tensor_scalar_mul(hp[:], ps[:], d9sq_sb[:, w:w + 1])
                    nc.sync.dma_start(
                        H_slice[w * WINDOW:(w + 1) * WINDOW, :C], hp[:])
                # lag-compensated early collective dispatch: quarter p's
                # slice is complete well before Pool's DGE stream reaches
                # quarter p+2, so the AllGather dispatch never stalls Pool
                # and its latency + cross-core skew hide under this hop's
                # remaining compute.
                if HF_next is not None:
                    gq = g + 1 - 2 * groups_per_part
                    if (gq >= 0 and gq % groups_per_part == 0
                            and gq // groups_per_part < cfg.n_parts - 2):
                        emit_allgather(HF_next, gq // groups_per_part)
                    if g + 1 == cfg.n_groups:
                        emit_allgather(HF_next, cfg.n_parts - 2)
                        emit_allgather(HF_next, cfg.n_parts - 1)

        for p in range(cfg.n_parts):
            emit_allgather(H_fulls[0], p)
        for k in range(cfg.K):
            hop_body(H_fulls[k], H_fulls[k + 1] if k + 1 < cfg.K else None)

        # ---- phase 3: log_softmax
        with tc.tile_pool(name="sm", bufs=4) as smp, \
             tc.tile_pool(name="smc", bufs=4) as smc:
            for w in range(nW):
                hp = smp.tile([WINDOW, C], BF16, tag="hp3")
                nc.sync.dma_start(hp[:], H_slice[w * WINDOW:(w + 1) * WINDOW, :C])
                h = smp.tile([WINDOW, C], F32, tag="h3")
                nc.vector.tensor_scalar_mul(h[:], hp[:], rdinv_sb[:, w:w + 1])
                nm = smc.tile([WINDOW, 1], F32, tag="nm")
                nc.vector.tensor_reduce(nm[:], h[:], mybir.AxisListType.X,
                                        ALU.max, negate=True)
                e = smp.tile([WINDOW, C], F32, tag="e3")
                se = smc.tile([WINDOW, 1], F32, tag="se")
                nc.scalar.activation(e[:], h[:], AF.Exp, bias=nm[:], accum_out=se[:])
                ls = smc.tile([WINDOW, 1], F32, tag="ls")
                nc.scalar.activation(ls[:], se[:], AF.Ln)
                o = smp.tile([WINDOW, C], F32, tag="o3")
                nc.vector.tensor_scalar(o[:], h[:], nm[:], ls[:],
                                        op0=ALU.add, op1=ALU.subtract)
                nc.sync.dma_start(out_d[w * WINDOW:(w + 1) * WINDOW, :], o[:])

    nc.compile()
    return nc


# test-harness knobs (not used by the grading path, which calls kernel() only)
PROFILE = False
LAST_EXEC_NS = None
LAST_TRACE = None

_BUILD_CACHE: dict = {}


def _get_compiled(cfg: Cfg, plan: Plan):
    key = (cfg.N, cfg.E, cfg.K, cfg.G, plan.tiles.tobytes())
    hit = _BUILD_CACHE.get(key)
    if hit is None:
        hit = build_kernel(cfg, plan)
        _BUILD_CACHE.clear()
        _BUILD_CACHE[key] = hit
    return hit


def kernel(x, W1, b1, W2, b2, edge_index):
    """Full (unsharded) inputs in, full [N, 64] log-softmax output out."""
    from concourse.bass_utils import run_bass_kernel_spmd

    x = np.asarray(x, dtype=np.float32)
    W1 = np.asarray(W1, dtype=np.float32)
    b1 = np.asarray(b1, dtype=np.float32)
    W2 = np.asarray(W2, dtype=np.float32)
    b2 = np.asarray(b2, dtype=np.float32)
    edge_index = np.asarray(edge_index)

    N, F = x.shape
    H = W1.shape[1]
    C = W2.shape[1]
    E = edge_index.shape[1]
    cfg = Cfg(N=N, E=E, F=F, H=H, C=C, K=10, alpha=0.1, n_cores=8)

    in_maps, plan = host_prep(cfg, x, W1, b1, W2, b2, edge_index)
    nc = _get_compiled(cfg, plan)
    res = run_bass_kernel_spmd(nc, in_maps, list(range(cfg.n_cores)),
                               trace=PROFILE)
    if PROFILE:
        global LAST_EXEC_NS, LAST_TRACE
        LAST_EXEC_NS = res.exec_time_ns
        LAST_TRACE = (res.instructions_and_trace or (None, None))[1]
    out = np.concatenate([res.results[i]["out"] for i in range(cfg.n_cores)],
                         axis=0)[:N]
    return np.ascontiguousarray(out, dtype=np.float32)
